# revision 1
# baseline (speedup 1.0000x reference)
"""Bayesian linear layer (reparameterized per-sample weights) on 8 trn2 NeuronCores.

y[b,o] = sum_i x[b,i] * (mu[o,i] + softplus(rho[o,i]) * eps_w[b,o,i])
         + bias_mu[o] + softplus(bias_rho[o]) * eps_b[b,o]

Sharding: data-parallel over batch. 8 cores x 32 samples. mu/rho replicated.

Per-core pipeline (the 128 MB eps_w shard dominates: ~360 GB/s HBM -> ~370 us):
  1. DMA eps_w[b] natural tiles [o=128p, i free] (contiguous, full BW)
  2. PE transpose 128x128 chunks -> PSUM   (gets i onto partitions)
  3. DVE single pass u = epsT (*) sigmaT, PSUM -> SBUF
  4. PE reduce-matmul, stationary = x[b, i_chunk] column (m=1), moving = u,
     float32r so fp32 data streams at 1 cycle/row; accumulates y2[b,:] in PSUM
  5. y_mu + bias terms precomputed into C[32,1024]; per-sample rows added via
     SBUF->SBUF accumulating DMA; one store of C to HBM.
"""

import numpy as np

import concourse.bass as bass
from concourse import bacc
import concourse.mybir as mybir
import concourse.tile as tile
from concourse.bass import ts
from concourse.bass_utils import run_bass_kernel_spmd
from concourse.masks import make_identity

FP32 = mybir.dt.float32
F32R = mybir.dt.float32r
AF = mybir.ActivationFunctionType

F = 1024          # feature dim (in == out)
N_CORES = 8
B_FULL = 256
NCH = F // 128    # 8 chunks of 128


def build_nc(BL: int, eps_bufs=3, pt_bufs=2, y2_bufs=2, u_bufs=4,
             ablate=()) -> bass.Bass:
    """Build the per-core Bass program for a local batch of BL samples."""
    nc = bacc.Bacc(None, target_bir_lowering=False)

    x_d = nc.declare_dram_parameter("x", [BL, F], FP32, isOutput=False)
    mu_d = nc.declare_dram_parameter("weight_mu", [F, F], FP32, isOutput=False)
    rho_d = nc.declare_dram_parameter("weight_rho", [F, F], FP32, isOutput=False)
    bmu_d = nc.declare_dram_parameter("bias_mu", [F], FP32, isOutput=False)
    brho_d = nc.declare_dram_parameter("bias_rho", [F], FP32, isOutput=False)
    epsw_d = nc.declare_dram_parameter("eps_w", [BL, F, F], FP32, isOutput=False)
    epsb_d = nc.declare_dram_parameter("eps_b", [BL, F], FP32, isOutput=False)
    y_d = nc.declare_dram_parameter("y", [BL, F], FP32, isOutput=True)

    # [b, (c p), i] -> [b, p, c, i]: partition p indexes o within chunk c
    epsw_t = epsw_d[:].rearrange("b (c p) i -> b p c i", p=128)
    mu_t = mu_d[:].rearrange("(c p) i -> p c i", p=128)
    rho_t = rho_d[:].rearrange("(c p) i -> p c i", p=128)

    with tile.TileContext(nc) as tc:
        with (
            tc.tile_pool(name="persist", bufs=1) as persist,
            tc.tile_pool(name="eps", bufs=eps_bufs) as epsp,
            tc.tile_pool(name="u", bufs=u_bufs) as up,
            tc.tile_pool(name="yrow", bufs=4) as yrowp,
            tc.tile_pool(name="pt", bufs=pt_bufs, space="PSUM") as ptp,
            tc.tile_pool(name="py2", bufs=y2_bufs, space="PSUM") as py2p,
        ):
            # ---------------- setup ----------------
            ident = persist.tile([128, 128], FP32)
            make_identity(nc, ident)

            # sigmaT[i, o] = softplus(rho[o, i]).T ; layout [128p(i in chunk k), k, o]
            sigT = persist.tile([128, NCH, F], FP32)
            rho_s = epsp.tile([128, NCH, F], FP32, tag="epst")
            nc.sync.dma_start(out=rho_s, in_=rho_t)
            for k in range(NCH):
                pt_k = ptp.tile([128, F], FP32, tag="pt_k")
                for c in range(NCH):
                    nc.tensor.transpose(
                        out=pt_k[:, ts(c, 128)],
                        in_=rho_s[:, c, ts(k, 128)],
                        identity=ident,
                    )
                sp_tmp = up.tile([128, F], FP32, tag="sp_tmp", name="sp_tmp")
                nc.scalar.activation(out=sp_tmp, in_=pt_k, func=AF.Exp)
                # softplus(x) = ln(1 + exp(x)); rho <= ~0 so no overflow
                nc.scalar.activation(out=sigT[:, k, :], in_=sp_tmp, func=AF.Ln, bias=1.0)

            tc.strict_bb_all_engine_barrier()

            # muT (setup only; slot returns to the eps pool afterwards)
            muT = epsp.tile([128, NCH, F], FP32, tag="epst")
            mu_s = epsp.tile([128, NCH, F], FP32, tag="epst")
            nc.sync.dma_start(out=mu_s, in_=mu_t)
            for k in range(NCH):
                pt_k = ptp.tile([128, F], FP32, tag="pt_k")
                for c in range(NCH):
                    nc.tensor.transpose(
                        out=pt_k[:, ts(c, 128)],
                        in_=mu_s[:, c, ts(k, 128)],
                        identity=ident,
                    )
                nc.scalar.copy(out=muT[:, k, :], in_=pt_k)

            tc.strict_bb_all_engine_barrier()

            # xT[i, b] ; layout [128p(i in chunk k), k, b]
            x_nat = persist.tile([BL, F], FP32)
            nc.sync.dma_start(out=x_nat, in_=x_d[:])
            xT = persist.tile([128, NCH, BL], FP32)
            xTr = persist.tile([128, NCH, BL], F32R)
            for k in range(NCH):
                ptx = ptp.tile([128, BL], FP32, tag="pt_k")
                nc.tensor.transpose(
                    out=ptx,
                    in_=x_nat[:, ts(k, 128)],
                    identity=ident[:BL, :BL],
                )
                nc.scalar.copy(out=xT[:, k, :], in_=ptx)
                nc.vector.tensor_copy(xTr[:, k, :], xT[:, k, :])

            tc.strict_bb_all_engine_barrier()

            # y_mu[b, o] = sum_i x[b,i] mu[o,i]  (full fp32 precision)
            ymu_ps = []
            for h in range(2):
                yp = ptp.tile([BL, 512], FP32, tag="pt_k")
                for k in range(NCH):
                    nc.tensor.matmul(
                        out=yp,
                        lhsT=xT[:, k, :],
                        rhs=muT[:, k, ts(h, 512)],
                        start=(k == 0),
                        stop=(k == NCH - 1),
                    )
                ymu_ps.append(yp)

            # C[b, o] = y_mu + bias_mu + softplus(bias_rho) * eps_b
            bmu_b = persist.tile([BL, F], FP32)
            nc.gpsimd.dma_start(
                out=bmu_b,
                in_=bass.AP(tensor=bmu_d, offset=0, ap=[[0, BL], [1, F]]),
            )
            sb_b = persist.tile([BL, F], FP32)
            nc.gpsimd.dma_start(
                out=sb_b,
                in_=bass.AP(tensor=brho_d, offset=0, ap=[[0, BL], [1, F]]),
            )
            nc.scalar.activation(out=sb_b, in_=sb_b, func=AF.Exp)
            nc.scalar.activation(out=sb_b, in_=sb_b, func=AF.Ln, bias=1.0)
            epsb_s = persist.tile([BL, F], FP32)
            nc.sync.dma_start(out=epsb_s, in_=epsb_d[:])

            C = persist.tile([BL, F], FP32)
            nc.vector.tensor_mul(C, sb_b, epsb_s)
            nc.vector.tensor_add(C, C, bmu_b)
            for h in range(2):
                nc.vector.tensor_add(C[:, ts(h, 512)], C[:, ts(h, 512)], ymu_ps[h])

            tc.strict_bb_all_engine_barrier()

            # ---------------- main loop over samples ----------------
            for b in range(BL):
                eb = epsp.tile([128, NCH, F], FP32, tag="epst")
                nc.sync.dma_start(out=eb, in_=epsw_t[b])

                y2 = [
                    py2p.tile([1, 512], FP32, tag=f"y2_{h}", name=f"y2_{h}")
                    for h in range(2)
                ]
                for k in range(NCH):
                    if "notrans" in ablate:
                        break
                    pt_k = ptp.tile([128, F], FP32, tag="pt_k")
                    for c in range(NCH):
                        nc.tensor.transpose(
                            out=pt_k[:, ts(c, 128)],
                            in_=eb[:, c, ts(k, 128)],
                            identity=ident,
                        )
                    if "nott" in ablate:
                        continue
                    u_k = up.tile([128, F], F32R)
                    nc.vector.tensor_mul(u_k, pt_k, sigT[:, k, :])
                    if "nomm" in ablate:
                        continue
                    for h in range(2):
                        nc.tensor.matmul(
                            out=y2[h],
                            lhsT=xTr[:, k, b : b + 1],
                            rhs=u_k[:, ts(h, 512)],
                            start=(k == 0),
                            stop=(k == NCH - 1),
                        )

                if not ablate:
                    yrow = yrowp.tile([1, F], FP32)
                    for h in range(2):
                        nc.scalar.copy(out=yrow[:, ts(h, 512)], in_=y2[h])
                    nc.gpsimd.dma_start(
                        out=C[b : b + 1, :], in_=yrow, accum_op=mybir.AluOpType.add
                    )

            nc.sync.dma_start(out=y_d[:], in_=C)

    nc.compile()
    return nc


_NC_CACHE: dict[int, bass.Bass] = {}


def _get_nc(BL: int) -> bass.Bass:
    if BL not in _NC_CACHE:
        _NC_CACHE[BL] = build_nc(BL)
    return _NC_CACHE[BL]


def kernel(x, weight_mu, weight_rho, bias_mu, bias_rho, eps_w, eps_b):
    B = x.shape[0]
    BL = B // N_CORES
    nc = _get_nc(BL)

    x = np.ascontiguousarray(np.asarray(x, dtype=np.float32))
    weight_mu = np.ascontiguousarray(np.asarray(weight_mu, dtype=np.float32))
    weight_rho = np.ascontiguousarray(np.asarray(weight_rho, dtype=np.float32))
    bias_mu = np.ascontiguousarray(np.asarray(bias_mu, dtype=np.float32))
    bias_rho = np.ascontiguousarray(np.asarray(bias_rho, dtype=np.float32))
    eps_w = np.ascontiguousarray(np.asarray(eps_w, dtype=np.float32))
    eps_b = np.ascontiguousarray(np.asarray(eps_b, dtype=np.float32))

    in_maps = []
    for i in range(N_CORES):
        sl = slice(i * BL, (i + 1) * BL)
        in_maps.append(
            {
                "x": x[sl],
                "weight_mu": weight_mu,
                "weight_rho": weight_rho,
                "bias_mu": bias_mu,
                "bias_rho": bias_rho,
                "eps_w": eps_w[sl],
                "eps_b": eps_b[sl],
            }
        )

    res = run_bass_kernel_spmd(nc, in_maps, core_ids=list(range(N_CORES)))
    return np.concatenate([r["y"] for r in res.results], axis=0)



# revision 11
# speedup vs baseline: 1.1312x; 1.1312x over previous
"""Bayesian linear layer (reparameterized per-sample weights) on 8 trn2 NeuronCores.

y[b,o] = sum_i x[b,i] * (mu[o,i] + softplus(rho[o,i]) * eps_w[b,o,i])
         + bias_mu[o] + softplus(bias_rho[o]) * eps_b[b,o]

Sharding: data-parallel over batch. 8 cores x 32 samples. mu/rho replicated.

Per-core pipeline (v2). The 128 MiB eps_w shard read dominates (~358 GB/s HBM
-> ~375 us floor); everything else must hide under it:
  1. SWDGE DMA casts eps_w[b] fp32->bf16 on the way in, "(p c)" o-layout so
     each partition reads one contiguous 32 KiB run per sample.
  2. DVE single 2x-mode pass u = eps (*) sigma in natural layout (all SBUF
     bf16 -> 2 elem/cycle/lane).
  3. PE transposes u in bf16 (1 cyc/row, half the fp32 cost) -> PSUM.
  4. Act (+ optionally DVE) evacuates PSUM -> SBUF bf16.
  5. PE reduce-matmul, stationary = x[b, i_chunk] bf16 column (m=1),
     accumulates y2[b,:] over the 8 i-chunks in PSUM.
  6. y_mu + bias terms in C (f-ordered); per-sample rows added via SBUF->SBUF
     accumulating DMA; one final unpermute + store.
No barriers: eps streaming starts at t=0 and setup hides under it.
"""

import numpy as np

import concourse.bass as bass
from concourse import bacc
import concourse.mybir as mybir
import concourse.tile as tile
from concourse.bass import ts
from concourse.bass_utils import run_bass_kernel_spmd
from concourse.masks import make_identity

FP32 = mybir.dt.float32
BF16 = mybir.dt.bfloat16
AF = mybir.ActivationFunctionType

F = 1024          # feature dim (in == out)
N_CORES = 8
B_FULL = 256
NCH = F // 128    # 8 chunks of 128


def build_nc(BL: int, eps_bufs=5, u_bufs=2, ut_bufs=4, pt_bufs=2, y2_bufs=2,
             evac_dve=0) -> bass.Bass:
    """Build the per-core Bass program for a local batch of BL samples.

    o-index layout: o = 8*p + c (partition p in 0..127, chunk c in 0..7), so
    a partition's 8 o-rows are contiguous in HBM.  Column order after the PE
    transpose ("f-order"): f = c*128 + p.  C and y2 are kept f-ordered until
    a single strided copy at the end restores natural o order.
    """
    nc = bacc.Bacc(None, target_bir_lowering=False)

    x_d = nc.declare_dram_parameter("x", [BL, F], FP32, isOutput=False)
    mu_d = nc.declare_dram_parameter("weight_mu", [F, F], FP32, isOutput=False)
    rho_d = nc.declare_dram_parameter("weight_rho", [F, F], FP32, isOutput=False)
    bmu_d = nc.declare_dram_parameter("bias_mu", [F], FP32, isOutput=False)
    brho_d = nc.declare_dram_parameter("bias_rho", [F], FP32, isOutput=False)
    epsw_d = nc.declare_dram_parameter("eps_w", [BL, F, F], FP32, isOutput=False)
    epsb_d = nc.declare_dram_parameter("eps_b", [BL, F], FP32, isOutput=False)
    y_d = nc.declare_dram_parameter("y", [BL, F], FP32, isOutput=True)

    # o = 8p + c: partition p covers o in [8p, 8p+8) -> 32 KiB contiguous.
    epsw_t = epsw_d[:].rearrange("b (p c) i -> b p c i", p=128)
    mu_t = mu_d[:].rearrange("(p c) i -> p c i", p=128)
    rho_t = rho_d[:].rearrange("(p c) i -> p c i", p=128)

    with tile.TileContext(nc) as tc:
        with (
            tc.tile_pool(name="persist", bufs=1) as persist,
            tc.tile_pool(name="setup", bufs=1) as setupp,
            tc.tile_pool(name="eps", bufs=eps_bufs) as epsp,
            tc.tile_pool(name="u", bufs=u_bufs) as up,
            tc.tile_pool(name="ut", bufs=ut_bufs) as utp,
            tc.tile_pool(name="yrow", bufs=2) as yrowp,
            tc.tile_pool(name="pt", bufs=pt_bufs, space="PSUM") as ptp,
            tc.tile_pool(name="py2", bufs=y2_bufs, space="PSUM") as py2p,
        ):
            # ---------------- setup (overlaps with eps streaming) ----------
            ident = persist.tile([128, 128], BF16)
            make_identity(nc, ident)

            # sigma in natural (p c) layout, bf16
            rho_s = setupp.tile([128, NCH, F], BF16, tag="stage", name="rho_s")
            nc.gpsimd.dma_start(out=rho_s, in_=rho_t)
            sig = persist.tile([128, NCH, F], BF16)
            # softplus(x) = ln(1 + exp(x)); rho <= ~0 so no overflow
            nc.scalar.activation(out=sig, in_=rho_s, func=AF.Exp)
            nc.scalar.activation(out=sig, in_=sig, func=AF.Ln, bias=1.0)

            # first eps DMAs issue here (program order on the SWDGE queue:
            # rho, then eps[0..1], then mu, ...)
            eps_tiles: dict[int, object] = {}

            def eps_dma(b):
                if b >= BL or b in eps_tiles:
                    return
                eb = epsp.tile([128, NCH, F], BF16, tag="epst", name=f"eb{b}")
                nc.gpsimd.dma_start(out=eb, in_=epsw_t[b])
                eps_tiles[b] = eb

            eps_dma(0)
            eps_dma(1)

            # muT (bf16): stage in (p c) layout, transpose on PE, evac via Act
            mu_s = setupp.tile([128, NCH, F], BF16, tag="stage", name="mu_s")
            nc.gpsimd.dma_start(out=mu_s, in_=mu_t)
            muT = persist.tile([128, NCH, F], BF16)
            for k in range(NCH):
                pt_k = ptp.tile([128, F], BF16, tag="pt_k", name=f"ptmu{k}")
                for c in range(NCH):
                    nc.tensor.transpose(
                        out=pt_k[:, ts(c, 128)],
                        in_=mu_s[:, c, ts(k, 128)],
                        identity=ident,
                    )
                nc.scalar.copy(out=muT[:, k, :], in_=pt_k)

            for b in range(2, 4):
                eps_dma(b)

            # xT[i, b] bf16 ; layout [128p(i in chunk k), k, b]
            x_nat = persist.tile([BL, F], FP32)
            nc.sync.dma_start(out=x_nat, in_=x_d[:])
            identf = persist.tile([BL, BL], FP32)
            make_identity(nc, identf)
            xT = persist.tile([128, NCH, BL], BF16)
            for k in range(NCH):
                ptx = ptp.tile([128, F], FP32, tag="pt_k", name=f"ptx{k}")
                nc.tensor.transpose(
                    out=ptx[:, :BL],
                    in_=x_nat[:, ts(k, 128)],
                    identity=identf,
                )
                nc.scalar.copy(out=xT[:, k, :], in_=ptx[:, :BL])

            # C (f-ordered) = y_mu + bias_mu + softplus(bias_rho) * eps_b
            bmu_b = persist.tile([BL, F], FP32)
            nc.gpsimd.dma_start(
                out=bmu_b,
                in_=bass.AP(tensor=bmu_d, offset=0, ap=[[0, BL], [1, F]]),
            )
            sb_b = persist.tile([BL, F], FP32)
            nc.gpsimd.dma_start(
                out=sb_b,
                in_=bass.AP(tensor=brho_d, offset=0, ap=[[0, BL], [1, F]]),
            )
            nc.scalar.activation(out=sb_b, in_=sb_b, func=AF.Exp)
            nc.scalar.activation(out=sb_b, in_=sb_b, func=AF.Ln, bias=1.0)
            epsb_s = persist.tile([BL, F], FP32)
            nc.sync.dma_start(out=epsb_s, in_=epsb_d[:])

            nc.vector.tensor_mul(sb_b, sb_b, epsb_s)
            nc.vector.tensor_add(sb_b, sb_b, bmu_b)
            # f-order it: C[b, f] with f = c*128 + p  <->  o = 8p + c
            C = persist.tile([BL, F], FP32)
            Cn_v = sb_b[:].rearrange("b (p c) -> b p c", p=128)
            for c in range(NCH):
                nc.vector.tensor_copy(C[:, ts(c, 128)], Cn_v[:, :, c])

            # y_mu[b, f] = sum_i x[b,i] mu[o(f),i]
            for h in range(2):
                yp = ptp.tile([BL, 512], FP32, tag="pt_k", name=f"ymu{h}")
                for k in range(NCH):
                    nc.tensor.matmul(
                        out=yp,
                        lhsT=xT[:, k, :],
                        rhs=muT[:, k, ts(h, 512)],
                        start=(k == 0),
                        stop=(k == NCH - 1),
                    )
                nc.vector.tensor_add(C[:, ts(h, 512)], C[:, ts(h, 512)], yp)

            # ---------------- main loop over samples ----------------
            for b in range(BL):
                eps_dma(b)          # no-op unless BL < 4 (tiny sim runs)
                eps_dma(b + 4)
                eb = eps_tiles.pop(b)

                # u = eps (*) sigma, one 2x-mode DVE op over all 8 chunks
                u = up.tile([128, NCH, F], BF16, tag="u", name=f"u{b}")
                nc.vector.tensor_mul(u, eb, sig)

                y2 = [
                    py2p.tile([1, 512], FP32, tag=f"y2_{h}", name=f"y2_{h}")
                    for h in range(2)
                ]
                for k in range(NCH):
                    pt_k = ptp.tile([128, F], BF16, tag="pt_k", name=f"pt{b}_{k}")
                    for c in range(NCH):
                        nc.tensor.transpose(
                            out=pt_k[:, ts(c, 128)],
                            in_=u[:, c, ts(k, 128)],
                            identity=ident,
                        )
                    ut_k = utp.tile([128, F], BF16, tag="ut", name=f"ut{b}_{k}")
                    if k < evac_dve:
                        nc.vector.tensor_copy(ut_k, pt_k)
                    else:
                        nc.scalar.copy(out=ut_k, in_=pt_k)
                    for h in range(2):
                        nc.tensor.matmul(
                            out=y2[h],
                            lhsT=xT[:, k, b : b + 1],
                            rhs=ut_k[:, ts(h, 512)],
                            start=(k == 0),
                            stop=(k == NCH - 1),
                        )

                yrow = yrowp.tile([1, F], FP32)
                for h in range(2):
                    nc.scalar.copy(out=yrow[:, ts(h, 512)], in_=y2[h])
                nc.gpsimd.dma_start(
                    out=C[b : b + 1, :], in_=yrow, accum_op=mybir.AluOpType.add
                )

            # undo the f-order permutation and store
            yout = persist.tile([BL, F], FP32)
            yout_v = yout[:].rearrange("b (p c) -> b p c", p=128)
            for c in range(NCH):
                nc.vector.tensor_copy(yout_v[:, :, c], C[:, ts(c, 128)])
            nc.sync.dma_start(out=y_d[:], in_=yout)

    nc.compile()
    return nc


_NC_CACHE: dict[int, bass.Bass] = {}


def _get_nc(BL: int) -> bass.Bass:
    if BL not in _NC_CACHE:
        _NC_CACHE[BL] = build_nc(BL)
    return _NC_CACHE[BL]


def kernel(x, weight_mu, weight_rho, bias_mu, bias_rho, eps_w, eps_b):
    B = x.shape[0]
    BL = B // N_CORES
    nc = _get_nc(BL)

    x = np.ascontiguousarray(np.asarray(x, dtype=np.float32))
    weight_mu = np.ascontiguousarray(np.asarray(weight_mu, dtype=np.float32))
    weight_rho = np.ascontiguousarray(np.asarray(weight_rho, dtype=np.float32))
    bias_mu = np.ascontiguousarray(np.asarray(bias_mu, dtype=np.float32))
    bias_rho = np.ascontiguousarray(np.asarray(bias_rho, dtype=np.float32))
    eps_w = np.ascontiguousarray(np.asarray(eps_w, dtype=np.float32))
    eps_b = np.ascontiguousarray(np.asarray(eps_b, dtype=np.float32))

    in_maps = []
    for i in range(N_CORES):
        sl = slice(i * BL, (i + 1) * BL)
        in_maps.append(
            {
                "x": x[sl],
                "weight_mu": weight_mu,
                "weight_rho": weight_rho,
                "bias_mu": bias_mu,
                "bias_rho": bias_rho,
                "eps_w": eps_w[sl],
                "eps_b": eps_b[sl],
            }
        )

    res = run_bass_kernel_spmd(nc, in_maps, core_ids=list(range(N_CORES)))
    return np.concatenate([r["y"] for r in res.results], axis=0)


# revision 19
# speedup vs baseline: 1.3061x; 1.1546x over previous
"""Bayesian linear layer (reparameterized per-sample weights) on 8 trn2 NeuronCores.

y[b,o] = sum_i x[b,i] * (mu[o,i] + softplus(rho[o,i]) * eps_w[b,o,i])
         + bias_mu[o] + softplus(bias_rho[o]) * eps_b[b,o]

Sharding: data-parallel over batch. 8 cores x 32 samples. mu/rho replicated.

Per-core pipeline (v2). The 128 MiB eps_w shard read dominates (~358 GB/s HBM
-> ~375 us floor); everything else must hide under it:
  1. SWDGE DMA casts eps_w[b] fp32->bf16 on the way in, "(p c)" o-layout so
     each partition reads one contiguous 32 KiB run per sample.
  2. DVE single 2x-mode pass u = eps (*) sigma in natural layout (all SBUF
     bf16 -> 2 elem/cycle/lane).
  3. PE transposes u in bf16 (1 cyc/row, half the fp32 cost) -> PSUM.
  4. Act (+ optionally DVE) evacuates PSUM -> SBUF bf16.
  5. PE reduce-matmul, stationary = x[b, i_chunk] bf16 column (m=1),
     accumulates y2[b,:] over the 8 i-chunks in PSUM.
  6. y_mu + bias terms in C (f-ordered); per-sample rows added via SBUF->SBUF
     accumulating DMA; one final unpermute + store.
No barriers: eps streaming starts at t=0 and setup hides under it.
"""

import numpy as np

import concourse.bass as bass
from concourse import bacc
import concourse.mybir as mybir
import concourse.tile as tile
from concourse.bass import ts
from concourse.bass_utils import run_bass_kernel_spmd
from concourse.masks import make_identity

FP32 = mybir.dt.float32
BF16 = mybir.dt.bfloat16
AF = mybir.ActivationFunctionType

F = 1024          # feature dim (in == out)
N_CORES = 8
B_FULL = 256
NCH = F // 128    # 8 chunks of 128


def build_nc(BL: int, eps_bufs=5, u_bufs=2, ut_bufs=4, pt_bufs=4, y2_bufs=1,
             evac_dve=3) -> bass.Bass:
    """Build the per-core Bass program for a local batch of BL samples.

    o-index layout: o = 8*p + c (partition p in 0..127, chunk c in 0..7), so
    a partition's 8 o-rows are contiguous in HBM.  Column order after the PE
    transpose ("f-order"): f = c*128 + p.  C and y2 are kept f-ordered until
    a single strided copy at the end restores natural o order.
    """
    nc = bacc.Bacc(None, target_bir_lowering=False)

    x_d = nc.declare_dram_parameter("x", [BL, F], FP32, isOutput=False)
    mu_d = nc.declare_dram_parameter("weight_mu", [F, F], FP32, isOutput=False)
    rho_d = nc.declare_dram_parameter("weight_rho", [F, F], FP32, isOutput=False)
    bmu_d = nc.declare_dram_parameter("bias_mu", [F], FP32, isOutput=False)
    brho_d = nc.declare_dram_parameter("bias_rho", [F], FP32, isOutput=False)
    epsw_d = nc.declare_dram_parameter("eps_w", [BL, F, F], FP32, isOutput=False)
    epsb_d = nc.declare_dram_parameter("eps_b", [BL, F], FP32, isOutput=False)
    y_d = nc.declare_dram_parameter("y", [BL, F], FP32, isOutput=True)

    # o = 8p + c: partition p covers o in [8p, 8p+8) -> 32 KiB contiguous.
    epsw_t = epsw_d[:].rearrange("b (p c) i -> b p c i", p=128)
    mu_t = mu_d[:].rearrange("(p c) i -> p c i", p=128)
    rho_t = rho_d[:].rearrange("(p c) i -> p c i", p=128)

    with tile.TileContext(nc) as tc:
        with (
            tc.tile_pool(name="persist", bufs=1) as persist,
            tc.tile_pool(name="setup", bufs=1) as setupp,
            tc.tile_pool(name="eps", bufs=eps_bufs) as epsp,
            tc.tile_pool(name="u", bufs=u_bufs) as up,
            tc.tile_pool(name="ut", bufs=ut_bufs) as utp,
            tc.tile_pool(name="yrow", bufs=2) as yrowp,
            tc.tile_pool(name="pt", bufs=pt_bufs, space="PSUM") as ptp,
            tc.tile_pool(name="ptf", bufs=2, space="PSUM") as ptfp,
            tc.tile_pool(name="py2", bufs=y2_bufs, space="PSUM") as py2p,
        ):
            # ---------------- setup (overlaps with eps streaming) ----------
            ident = persist.tile([128, 128], BF16)
            make_identity(nc, ident)

            # sigma in natural (p c) layout, bf16
            rho_s = setupp.tile([128, NCH, F], BF16, tag="stage", name="rho_s")
            nc.gpsimd.dma_start(out=rho_s, in_=rho_t)
            sig = persist.tile([128, NCH, F], BF16)
            # softplus(x) = ln(1 + exp(x)); rho <= ~0 so no overflow
            nc.scalar.activation(out=sig, in_=rho_s, func=AF.Exp)
            nc.scalar.activation(out=sig, in_=sig, func=AF.Ln, bias=1.0)

            # first eps DMAs issue here (program order on the SWDGE queue:
            # rho, then eps[0..1], then mu, ...)
            eps_tiles: dict[int, object] = {}

            def eps_dma(b):
                if b >= BL or b in eps_tiles:
                    return
                eb = epsp.tile([128, NCH, F], BF16, tag="epst", name=f"eb{b}")
                nc.gpsimd.dma_start(out=eb, in_=epsw_t[b])
                eps_tiles[b] = eb

            eps_dma(0)
            eps_dma(1)

            # muT (bf16): stage in (p c) layout, transpose on PE, evac via Act
            mu_s = setupp.tile([128, NCH, F], BF16, tag="stage", name="mu_s")
            nc.gpsimd.dma_start(out=mu_s, in_=mu_t)
            muT = persist.tile([128, NCH, F], BF16)
            for k in range(NCH):
                pt_k = ptfp.tile([128, F], BF16, tag="ptf", name=f"ptmu{k}")
                for c in range(NCH):
                    nc.tensor.transpose(
                        out=pt_k[:, ts(c, 128)],
                        in_=mu_s[:, c, ts(k, 128)],
                        identity=ident,
                    )
                nc.scalar.copy(out=muT[:, k, :], in_=pt_k)

            for b in range(2, 4):
                eps_dma(b)

            # xT[i, b] bf16 ; layout [128p(i in chunk k), k, b]
            x_nat = persist.tile([BL, F], FP32)
            nc.sync.dma_start(out=x_nat, in_=x_d[:])
            x_bf = persist.tile([BL, F], BF16)
            nc.vector.tensor_copy(x_bf, x_nat)
            xT = persist.tile([128, NCH, BL], BF16)
            for k in range(NCH):
                ptx = ptfp.tile([128, F], BF16, tag="ptf", name=f"ptx{k}")
                nc.tensor.transpose(
                    out=ptx[:, :BL],
                    in_=x_bf[:, ts(k, 128)],
                    identity=ident[:BL, :BL],
                )
                nc.scalar.copy(out=xT[:, k, :], in_=ptx[:, :BL])

            # C (f-ordered) = y_mu + bias_mu + softplus(bias_rho) * eps_b
            bmu_b = persist.tile([BL, F], FP32)
            nc.gpsimd.dma_start(
                out=bmu_b,
                in_=bass.AP(tensor=bmu_d, offset=0, ap=[[0, BL], [1, F]]),
            )
            sb_b = persist.tile([BL, F], FP32)
            nc.gpsimd.dma_start(
                out=sb_b,
                in_=bass.AP(tensor=brho_d, offset=0, ap=[[0, BL], [1, F]]),
            )
            nc.scalar.activation(out=sb_b, in_=sb_b, func=AF.Exp)
            nc.scalar.activation(out=sb_b, in_=sb_b, func=AF.Ln, bias=1.0)
            epsb_s = persist.tile([BL, F], FP32)
            nc.sync.dma_start(out=epsb_s, in_=epsb_d[:])

            nc.vector.tensor_mul(sb_b, sb_b, epsb_s)
            nc.vector.tensor_add(sb_b, sb_b, bmu_b)
            # f-order it: C[b, f] with f = c*128 + p  <->  o = 8p + c
            C = persist.tile([BL, F], FP32)
            Cn_v = sb_b[:].rearrange("b (p c) -> b p c", p=128)
            for c in range(NCH):
                nc.vector.tensor_copy(C[:, ts(c, 128)], Cn_v[:, :, c])

            # y_mu[b, f] = sum_i x[b,i] mu[o(f),i]
            for h in range(2):
                yp = ptfp.tile([BL, 512], FP32, tag="ptf", name=f"ymu{h}")
                for k in range(NCH):
                    nc.tensor.matmul(
                        out=yp,
                        lhsT=xT[:, k, :],
                        rhs=muT[:, k, ts(h, 512)],
                        start=(k == 0),
                        stop=(k == NCH - 1),
                    )
                nc.vector.tensor_add(C[:, ts(h, 512)], C[:, ts(h, 512)], yp)

            # ---------------- main loop over samples ----------------
            for b in range(BL):
                eps_dma(b)          # no-op unless BL < 4 (tiny sim runs)
                eps_dma(b + 4)
                eb = eps_tiles.pop(b)

                # u = eps (*) sigma, one 2x-mode DVE op over all 8 chunks
                u = up.tile([128, NCH, F], BF16, tag="u", name=f"u{b}")
                nc.vector.tensor_mul(u, eb, sig)

                y2 = [
                    py2p.tile([1, 512], FP32, tag=f"y2_{h}", name=f"y2_{h}")
                    for h in range(2)
                ]
                for k in range(NCH):
                    pt_k = ptp.tile([128, F], BF16, tag="pt_k", name=f"pt{b}_{k}")
                    for c in range(NCH):
                        nc.tensor.transpose(
                            out=pt_k[:, ts(c, 128)],
                            in_=u[:, c, ts(k, 128)],
                            identity=ident,
                        )
                    ut_k = utp.tile([128, F], BF16, tag="ut", name=f"ut{b}_{k}")
                    if k < evac_dve:
                        nc.vector.tensor_copy(ut_k, pt_k)
                    else:
                        nc.scalar.copy(out=ut_k, in_=pt_k)
                    for h in range(2):
                        nc.tensor.matmul(
                            out=y2[h],
                            lhsT=xT[:, k, b : b + 1],
                            rhs=ut_k[:, ts(h, 512)],
                            start=(k == 0),
                            stop=(k == NCH - 1),
                        )

                yrow = yrowp.tile([1, F], FP32)
                for h in range(2):
                    nc.scalar.copy(out=yrow[:, ts(h, 512)], in_=y2[h])
                nc.gpsimd.dma_start(
                    out=C[b : b + 1, :], in_=yrow, accum_op=mybir.AluOpType.add
                )

            # undo the f-order permutation and store
            yout = persist.tile([BL, F], FP32)
            yout_v = yout[:].rearrange("b (p c) -> b p c", p=128)
            for c in range(NCH):
                nc.vector.tensor_copy(yout_v[:, :, c], C[:, ts(c, 128)])
            nc.sync.dma_start(out=y_d[:], in_=yout)

    nc.compile()
    return nc


_NC_CACHE: dict[int, bass.Bass] = {}


def _get_nc(BL: int) -> bass.Bass:
    if BL not in _NC_CACHE:
        _NC_CACHE[BL] = build_nc(BL)
    return _NC_CACHE[BL]


def kernel(x, weight_mu, weight_rho, bias_mu, bias_rho, eps_w, eps_b):
    B = x.shape[0]
    BL = B // N_CORES
    nc = _get_nc(BL)

    x = np.ascontiguousarray(np.asarray(x, dtype=np.float32))
    weight_mu = np.ascontiguousarray(np.asarray(weight_mu, dtype=np.float32))
    weight_rho = np.ascontiguousarray(np.asarray(weight_rho, dtype=np.float32))
    bias_mu = np.ascontiguousarray(np.asarray(bias_mu, dtype=np.float32))
    bias_rho = np.ascontiguousarray(np.asarray(bias_rho, dtype=np.float32))
    eps_w = np.ascontiguousarray(np.asarray(eps_w, dtype=np.float32))
    eps_b = np.ascontiguousarray(np.asarray(eps_b, dtype=np.float32))

    in_maps = []
    for i in range(N_CORES):
        sl = slice(i * BL, (i + 1) * BL)
        in_maps.append(
            {
                "x": x[sl],
                "weight_mu": weight_mu,
                "weight_rho": weight_rho,
                "bias_mu": bias_mu,
                "bias_rho": bias_rho,
                "eps_w": eps_w[sl],
                "eps_b": eps_b[sl],
            }
        )

    res = run_bass_kernel_spmd(nc, in_maps, core_ids=list(range(N_CORES)))
    return np.concatenate([r["y"] for r in res.results], axis=0)
